# revision 1
# baseline (speedup 1.0000x reference)
"""Trainium2 Bass kernel: GSpade node embedding.

Computation (see reference):
  - bidirectional tanh-RNN (hidden 512/dir) over T=32768 tokens grouped into
    N=2048 contiguous ragged segments (sorted group ids in `masks`)
  - mean-pool hidden states per segment -> pooled [N, 1024]
  - out = [x @ Wx.T + bx | pooled]  -> [N, 2048]

Sharding (8 NeuronCores, SPMD single program):
  - cores 0-3: forward RNN, cores 4-7: backward RNN.  Segments are sorted by
    length (desc) and striped: scan-stripe c (= core % 4) owns segments with
    sorted rank 4i+c, i.e. 512 "lanes" per core.  Forward core c and backward
    core c+4 own the same segments, so pooled = [pooled_f | pooled_b] splits
    column-wise with no cross-core traffic.
  - each core also computes 256 rows of the x-projection.

Per-core scan layout: hidden state h is kept transposed [feature(4x128p), lane]
so the PE contraction dim (features) stays on partitions across steps - no
per-step transpose.  Lanes are end-aligned to a shared schedule L_i (max of the
4 stripes at rank i): a lane's segment is zero-PREFIXED, so h stays exactly 0
(tanh(0 + W@0) = 0, bias is masked) until its first real token, and every lane
retires exactly at step L_i.  Active lane count N(t) = #{L_i > t} shrinks with
t, and the sorted layout makes the active set a prefix -> pure slicing, no
masking, ~0.1% padded work.

Per step t (N = N(t) lanes), accumulated in PSUM [128, 4x512] (4 banks):
  psum[jc] = W_ih.T[:,jc] @ x_t  (input proj, raw tokens, start=True)
           + b[jc] (x) mask_t    (rank-1 matmul; mask kills zero-prefix lanes)
           + sum_kc W_hh.T[kc,jc] @ h[kc]   (16 matmuls)
  h' = tanh(psum)                (1-2 fused ACT ops over the 4 banks)
  acc += h'                      (DVE, mean-pool accumulator, sliced to N(t))
"""

import ml_dtypes
import numpy as np

import concourse.bacc as bacc
import concourse.mybir as mybir
from concourse.tile import TileContext
from concourse.bass_utils import run_bass_kernel_spmd

FP32 = mybir.dt.float32
F32R = mybir.dt.float32r
BF16 = mybir.dt.bfloat16
Tanh = mybir.ActivationFunctionType.Tanh

N_GROUPS = 2048
D_SEQ = 128
H = 512           # hidden per direction
HC = 4            # hidden chunks of 128
D_PROJ = 1024
N_CORES = 8
LANES = 512       # segments per scan core
XROWS = N_GROUPS // N_CORES  # x-projection rows per core

_program_cache: dict = {}


def _dma_chunks(nt, target=2048):
    """Group steps into DMA chunks of ~target columns. Returns [(t0, t1), ...]."""
    chunks = []
    t0 = 0
    cols = 0
    for t, n in enumerate(nt):
        if cols > 0 and cols + n > target:
            chunks.append((t0, t))
            t0, cols = t, 0
        cols += n
    chunks.append((t0, len(nt)))
    return chunks


def _build_program(nt, nt_true):
    """Build + compile the SPMD program. nt = even-padded N(t) (fp32r matmul
    needs even free dims); nt_true = real active-lane counts (acc slicing)."""
    nt = list(nt)
    nt_true = list(nt_true)
    steps = len(nt)
    off = np.concatenate([[0], np.cumsum(nt)]).astype(int)
    S = int(off[-1])

    nc = bacc.Bacc("TRN2", target_bir_lowering=False, debug=False,
                   num_devices=N_CORES)

    xtok_d = nc.dram_tensor("xtok", [128, S], BF16, kind="ExternalInput")
    mrow_d = nc.dram_tensor("mrow", [1, S], F32R, kind="ExternalInput")
    whhT_d = nc.dram_tensor("whhT", [H, H], BF16, kind="ExternalInput")
    wihT_d = nc.dram_tensor("wihT", [D_SEQ, H], BF16, kind="ExternalInput")
    brow_d = nc.dram_tensor("brow", [1, H], F32R, kind="ExternalInput")
    invl_d = nc.dram_tensor("invl", [128, LANES], FP32, kind="ExternalInput")
    xT_d = nc.dram_tensor("xT", [H, XROWS], F32R, kind="ExternalInput")
    wxT_d = nc.dram_tensor("wxT", [H, D_PROJ], F32R, kind="ExternalInput")
    bxrow_d = nc.dram_tensor("bxrow", [1, D_PROJ], F32R, kind="ExternalInput")
    ones_d = nc.dram_tensor("ones", [1, 128], F32R, kind="ExternalInput")

    xp_d = nc.dram_tensor("xp", [XROWS, D_PROJ], FP32, kind="ExternalOutput")
    pooledT_d = nc.dram_tensor("pooledT", [H, LANES], FP32, kind="ExternalOutput")

    with TileContext(nc) as tc:
        with (
            tc.tile_pool(name="sb", bufs=1) as sb,
            tc.tile_pool(name="ps", bufs=2, space="PSUM") as psp,
        ):
            # ---- persistent SBUF tiles + loads ----
            wx_sb = [sb.tile([128, D_PROJ], F32R, tag=f"wx{k}", name=f"wx{k}") for k in range(4)]
            xT_sb = [sb.tile([128, XROWS], F32R, tag=f"xT{k}", name=f"xT{k}") for k in range(4)]
            ones_sb = sb.tile([1, 128], F32R, tag="ones", name="ones")
            bx_sb = sb.tile([1, D_PROJ], F32R, tag="bx", name="bx")
            for k in range(4):
                nc.sync.dma_start(out=wx_sb[k][:, :], in_=wxT_d[k * 128:(k + 1) * 128, :])
                nc.sync.dma_start(out=xT_sb[k][:, :], in_=xT_d[k * 128:(k + 1) * 128, :])
            nc.sync.dma_start(out=ones_sb[:, :], in_=ones_d[:, :])
            nc.sync.dma_start(out=bx_sb[:, :], in_=bxrow_d[:, :])

            wh_sb = [sb.tile([128, H], BF16, tag=f"wh{k}", name=f"wh{k}") for k in range(HC)]
            wih_sb = sb.tile([128, H], BF16, tag="wih", name="wih")
            brow_sb = sb.tile([1, H], F32R, tag="brow", name="brow")
            invl_sb = sb.tile([128, LANES], FP32, tag="invl", name="invl")
            mrow_sb = sb.tile([1, S], F32R, tag="mrow", name="mrow")
            xtok_sb = sb.tile([128, S], BF16, tag="xtok", name="xtok")
            for k in range(HC):
                nc.sync.dma_start(out=wh_sb[k][:, :], in_=whhT_d[k * 128:(k + 1) * 128, :])
            nc.sync.dma_start(out=wih_sb[:, :], in_=wihT_d[:, :])
            nc.sync.dma_start(out=brow_sb[:, :], in_=brow_d[:, :])
            nc.sync.dma_start(out=invl_sb[:, :], in_=invl_d[:, :])
            nc.sync.dma_start(out=mrow_sb[:, :], in_=mrow_d[:, :])
            for (t0, t1) in _dma_chunks(nt):
                a, b = int(off[t0]), int(off[t1])
                nc.sync.dma_start(out=xtok_sb[:, a:b], in_=xtok_d[:, a:b])

            # scan state
            h_sb = [sb.tile([128, HC * H], BF16, tag=f"h{p}", name=f"h{p}") for p in range(2)]
            acc_sb = sb.tile([128, HC * H], FP32, tag="acc", name="acc")

            # acc = 0, routed through ACT tanh so the tanh table set loads
            # up-front (overlapping the x-proj phase) instead of stalling
            # the first scan step.
            nc.vector.memset(acc_sb[:, :], 0.0)
            nc.scalar.activation(acc_sb[:, :], acc_sb[:, :], Tanh)

            # ---- phase A: x projection (also the PE/HAM warm-up) ----
            xp_sb = [sb.tile([128, D_PROJ], FP32, tag=f"xp{b}", name=f"xpsb{b}") for b in range(2)]
            for bc in range(2):
                ps = psp.tile([128, HC * H], FP32, tag="ps", name="ps")
                for jh in range(2):
                    o = ps[:, jh * H:jh * H + H]
                    nc.tensor.matmul(o, ones_sb[0:1, :], bx_sb[0:1, jh * H:(jh + 1) * H],
                                     start=True, stop=False)
                    for kc in range(4):
                        nc.tensor.matmul(o, xT_sb[kc][:, bc * 128:(bc + 1) * 128],
                                         wx_sb[kc][:, jh * H:(jh + 1) * H],
                                         start=False, stop=(kc == 3))
                    nc.vector.tensor_copy(xp_sb[bc][:, jh * H:(jh + 1) * H], o)
                nc.sync.dma_start(out=xp_d[bc * 128:(bc + 1) * 128, :], in_=xp_sb[bc][:, :])

            # ---- scan ----
            for t in range(steps):
                n = nt[t]
                na = nt_true[t]
                a = int(off[t])
                hr = h_sb[(t + 1) % 2]   # state produced by step t-1
                hw = h_sb[t % 2]         # state produced by this step
                xcur = xtok_sb[:, a:a + n]
                mcur = mrow_sb[0:1, a:a + n]
                ps = psp.tile([128, HC * H], FP32, tag="ps", name="ps")

                # input projection + masked bias (independent of h)
                for jc in range(HC):
                    o = ps[:, jc * H:jc * H + n]
                    nc.tensor.matmul(o, wih_sb[:, jc * 128:(jc + 1) * 128], xcur,
                                     start=True, stop=False)
                for jc in range(HC):
                    o = ps[:, jc * H:jc * H + n]
                    nc.tensor.matmul(o, brow_sb[0:1, jc * 128:(jc + 1) * 128], mcur,
                                     start=False, stop=(t == 0))
                ps3 = ps.rearrange("p (c n) -> p c n", c=HC)
                hw3 = hw.rearrange("p (c n) -> p c n", c=HC)
                acc3 = acc_sb.rearrange("p (c n) -> p c n", c=HC)

                if t > 0:
                    # hidden recurrence, k-chunk outer so each j-bank finishes late
                    # but the 8 h-independent matmuls above cover the tanh latency
                    for kc in range(HC):
                        hk = hr[:, kc * H:kc * H + n]
                        for jc in range(HC):
                            nc.tensor.matmul(ps[:, jc * H:jc * H + n],
                                             wh_sb[kc][:, jc * 128:(jc + 1) * 128], hk,
                                             start=False, stop=(kc == HC - 1))
                            if kc == HC - 1 and n >= 256 and jc == 1:
                                nc.scalar.activation(hw3[:, 0:2, 0:n], ps3[:, 0:2, 0:n], Tanh)
                    if n >= 256:
                        nc.scalar.activation(hw3[:, 2:4, 0:n], ps3[:, 2:4, 0:n], Tanh)
                    else:
                        nc.scalar.activation(hw3[:, 0:4, 0:n], ps3[:, 0:4, 0:n], Tanh)
                else:
                    nc.scalar.activation(hw3[:, 0:4, 0:n], ps3[:, 0:4, 0:n], Tanh)

                if n >= 256:
                    nc.vector.tensor_add(acc3[:, 0:2, 0:na], acc3[:, 0:2, 0:na], hw3[:, 0:2, 0:na])
                    nc.vector.tensor_add(acc3[:, 2:4, 0:na], acc3[:, 2:4, 0:na], hw3[:, 2:4, 0:na])
                else:
                    nc.vector.tensor_add(acc3[:, 0:4, 0:na], acc3[:, 0:4, 0:na], hw3[:, 0:4, 0:na])

            # ---- finalize: pooledT[jc] = acc[jc] * (1/len) ----
            for jc in range(HC):
                po = sb.tile([128, LANES], FP32, tag=f"po{jc}", name=f"po{jc}")
                nc.vector.tensor_mul(po[:, :], acc_sb[:, jc * H:(jc + 1) * H], invl_sb[:, :])
                nc.sync.dma_start(out=pooledT_d[jc * 128:(jc + 1) * 128, :], in_=po[:, :])

    nc.compile()
    return nc


def _get_program(nt, nt_true):
    key = (tuple(nt), tuple(nt_true))
    if key not in _program_cache:
        _program_cache[key] = _build_program(nt, nt_true)
    return _program_cache[key]


def _prepare(x, seqs, masks, W_ih_f, W_hh_f, b_f, W_ih_b, W_hh_b, b_b, Wx, bx):
    x = np.asarray(x, np.float32)
    seqs = np.asarray(seqs, np.float32)
    masks = np.asarray(masks).astype(np.int64)

    T = seqs.shape[0]

    # ---- segment geometry (host) ----
    lens = np.bincount(masks, minlength=N_GROUPS).astype(np.int64)
    starts_all = np.concatenate([[0], np.cumsum(lens)[:-1]])
    order = np.argsort(-lens, kind="stable")          # groups sorted by len desc
    sl = lens[order]
    L = sl[0::4].astype(np.int64)                     # shared lane schedule (512)
    steps = int(L[0])
    nt_true = [int((L > t).sum()) for t in range(steps)]
    nt = [(n + 1) // 2 * 2 for n in nt_true]          # fp32r: even matmul widths
    off_true = np.concatenate([[0], np.cumsum(nt_true)]).astype(int)
    off = np.concatenate([[0], np.cumsum(nt)]).astype(int)
    S = int(off[-1])

    def pad_stream(flat2d):
        # [rows, S_true] -> [rows, S] inserting one zero col per odd step
        out = np.zeros((flat2d.shape[0], S), flat2d.dtype)
        for t in range(steps):
            out[:, off[t]:off[t] + nt_true[t]] = flat2d[:, off_true[t]:off_true[t] + nt_true[t]]
        return out

    # active/real masks per (step, lane); active lanes are a prefix each step
    t_grid = np.arange(steps)[:, None]
    active = t_grid < L[None, :]                      # [steps, LANES]

    seqs_pad = np.vstack([np.zeros((1, D_SEQ), np.float32), seqs])

    gid = [order[c::4] for c in range(4)]
    in_maps = []
    per_stripe = {}
    for c in range(4):
        lens_c = lens[gid[c]]
        starts_c = starts_all[gid[c]]
        pre = (L - lens_c)[None, :]                   # zero-prefix length
        real = active & (t_grid >= pre)
        pos = t_grid - pre
        idx_f = np.where(real, starts_c[None, :] + pos, -1)
        idx_b = np.where(real, starts_c[None, :] + lens_c[None, :] - 1 - pos, -1)
        real_flat = real[active]
        xtok_f = pad_stream(np.ascontiguousarray(seqs_pad[idx_f[active] + 1].T))
        xtok_b = pad_stream(np.ascontiguousarray(seqs_pad[idx_b[active] + 1].T))
        mrow = pad_stream(real_flat.astype(np.float32)[None, :])
        invl = np.ascontiguousarray(
            np.broadcast_to((1.0 / lens_c).astype(np.float32)[None, :], (128, LANES)))
        per_stripe[c] = (xtok_f, xtok_b, mrow, invl)

    ones = np.ones((1, 128), np.float32)
    wxT = np.ascontiguousarray(np.asarray(Wx, np.float32).T)
    bxr = np.asarray(bx, np.float32)[None, :]
    for core in range(N_CORES):
        c = core % 4
        fwd = core < 4
        xtok_f, xtok_b, mrow, invl = per_stripe[c]
        W_ih = W_ih_f if fwd else W_ih_b
        W_hh = W_hh_f if fwd else W_hh_b
        b = b_f if fwd else b_b
        in_maps.append({
            "xtok": (xtok_f if fwd else xtok_b).astype(ml_dtypes.bfloat16),
            "mrow": mrow,
            "whhT": np.ascontiguousarray(np.asarray(W_hh, np.float32).T).astype(ml_dtypes.bfloat16),
            "wihT": np.ascontiguousarray(np.asarray(W_ih, np.float32).T).astype(ml_dtypes.bfloat16),
            "brow": np.asarray(b, np.float32)[None, :],
            "invl": invl,
            "xT": np.ascontiguousarray(
                x[core * XROWS:(core + 1) * XROWS, :].T),
            "wxT": wxT,
            "bxrow": bxr,
            "ones": ones,
        })

    return (nt, nt_true), in_maps, gid


def _assemble(res, gid):
    out = np.empty((N_GROUPS, 2 * D_PROJ), np.float32)
    for core in range(N_CORES):
        out[core * XROWS:(core + 1) * XROWS, :D_PROJ] = res[core]["xp"]
    for c in range(4):
        out[gid[c], D_PROJ:D_PROJ + H] = res[c]["pooledT"].T
        out[gid[c], D_PROJ + H:] = res[c + 4]["pooledT"].T
    return out


def kernel(**inputs):
    (nt, nt_true), in_maps, gid = _prepare(**inputs)
    nc = _get_program(nt, nt_true)
    res = run_bass_kernel_spmd(nc, in_maps, list(range(N_CORES))).results
    return _assemble(res, gid)



# revision 5
# speedup vs baseline: 1.6158x; 1.6158x over previous
"""Trainium2 Bass kernel: GSpade node embedding (fp8 DoubleRow scan).

Computation (see reference):
  - bidirectional tanh-RNN (hidden 512/dir) over T=32768 tokens grouped into
    N=2048 contiguous ragged segments (sorted group ids in `masks`)
  - mean-pool hidden states per segment -> pooled [N, 1024]
  - out = [x @ Wx.T + bx | pooled]  -> [N, 2048]

Sharding (8 NeuronCores, SPMD single program):
  cores 0-3 forward RNN, 4-7 backward.  Segments sorted by length (desc) and
  striped: stripe c (= core % 4) owns segments ranked 4i+c.  Each core also
  computes 256 rows of the x-projection.

Per-core scan: h kept transposed [feature(4x128p), lane].  Lanes end-aligned
to the shared schedule L_i (max of the 4 stripes at rank i) with zero-prefix,
so h stays exactly 0 until the first real token (bias is masked via a second
matmul k-slab, see below).  Lanes are split into two interleaved blocks
(even/odd rank) that alternate per step: while ACT tanh's block A, the PE runs
block B's matmuls - hides the tanh latency in the serial recurrence chain.

All scan matmuls are fp8e4 DoubleRow (2 k-slabs per instr, 0.5 PE cycles per
output column):
  - input proj: slab0 = 64*W_ih.T vs token slab, slab1 = [64*b; 0..] vs
    [mask; 0..] slab -> psum = 64*(W_ih.T x + b*mask) in one pass
  - recurrence: 2 k-pair DR matmuls per j-chunk vs fp8 h of prev step
  - mean-pool:  [I|0] / [0|I] identity DR matmuls accumulate h into a
    PSUM accumulator (start at t=1, stop after the last step)
ACT: h = tanh(psum * 1/64) -> fp8, one instr per block-step.
PSUM: 2 banks ps + 2 banks pool-acc per block = all 8 banks.
Finalize: DVE multiplies acc by 1/len, DMA out.
"""

import ml_dtypes
import numpy as np

import concourse.bacc as bacc
import concourse.mybir as mybir
from concourse.tile import TileContext
from concourse.bass_utils import run_bass_kernel_spmd

FP32 = mybir.dt.float32
F32R = mybir.dt.float32r
FP8 = mybir.dt.float8e4
NP8 = ml_dtypes.float8_e4m3
Tanh = mybir.ActivationFunctionType.Tanh
DR = mybir.MatmulPerfMode.DoubleRow

N_GROUPS = 2048
D_SEQ = 128
H = 512           # hidden per direction
HC = 4            # hidden chunks of 128
D_PROJ = 1024
N_CORES = 8
LANES = 512       # segments per scan core
BL = 256          # lanes per block
XROWS = N_GROUPS // N_CORES  # x-projection rows per core
LAM = 64.0        # fp8 weight scale, undone by ACT scale=1/LAM

_program_cache: dict = {}


def _dma_chunks(widths, target=2048):
    """Group per-step stream widths into DMA chunks of ~target columns."""
    chunks, t0, cols = [], 0, 0
    for t, w in enumerate(widths):
        if cols > 0 and cols + w > target:
            chunks.append((t0, t))
            t0, cols = t, 0
        cols += w
    chunks.append((t0, len(widths)))
    return chunks


def _build_program(sched):
    """sched = (wtrue[2][steps], wpad[2][steps]) per block."""
    wtrue, wpad = sched
    steps = len(wtrue[0])
    # stream column offsets per block (2 slabs per step)
    off = []
    for b in range(2):
        o = np.concatenate([[0], np.cumsum([2 * w for w in wpad[b]])]).astype(int)
        off.append(o)
    S2 = [int(off[b][-1]) for b in range(2)]

    nc = bacc.Bacc("TRN2", target_bir_lowering=False, debug=False,
                   num_devices=N_CORES)

    xtok_d = [nc.dram_tensor(f"xtok{b}", [D_SEQ, max(S2[b], 2)], FP8,
                             kind="ExternalInput") for b in range(2)]
    wih_d = nc.dram_tensor("wih", [D_SEQ, 2 * H], FP8, kind="ExternalInput")
    whp_d = nc.dram_tensor("whp", [128, 2 * 2 * H], FP8, kind="ExternalInput")
    ident_d = nc.dram_tensor("ident", [128, 2 * BL], FP8, kind="ExternalInput")
    invl_d = nc.dram_tensor("invl", [128, LANES], FP32, kind="ExternalInput")
    xT_d = nc.dram_tensor("xT", [H, XROWS], F32R, kind="ExternalInput")
    wxT_d = nc.dram_tensor("wxT", [H, D_PROJ], F32R, kind="ExternalInput")
    bxrow_d = nc.dram_tensor("bxrow", [1, D_PROJ], F32R, kind="ExternalInput")
    ones_d = nc.dram_tensor("ones", [1, 128], F32R, kind="ExternalInput")

    xp_d = nc.dram_tensor("xp", [XROWS, D_PROJ], FP32, kind="ExternalOutput")
    pooledT_d = [nc.dram_tensor(f"pooledT{b}", [H, BL], FP32,
                                kind="ExternalOutput") for b in range(2)]

    with TileContext(nc) as tc:
        with (
            tc.tile_pool(name="sb", bufs=1) as sb,
            tc.tile_pool(name="ps", bufs=1, space="PSUM") as psp,
        ):
            # ---- persistent SBUF tiles + loads ----
            wx_sb = [sb.tile([128, D_PROJ], F32R, tag=f"wx{k}", name=f"wx{k}") for k in range(4)]
            xT_sb = [sb.tile([128, XROWS], F32R, tag=f"xT{k}", name=f"xT{k}") for k in range(4)]
            ones_sb = sb.tile([1, 128], F32R, tag="ones", name="ones")
            bx_sb = sb.tile([1, D_PROJ], F32R, tag="bx", name="bx")
            for k in range(4):
                nc.sync.dma_start(out=wx_sb[k][:, :], in_=wxT_d[k * 128:(k + 1) * 128, :])
                nc.sync.dma_start(out=xT_sb[k][:, :], in_=xT_d[k * 128:(k + 1) * 128, :])
            nc.sync.dma_start(out=ones_sb[:, :], in_=ones_d[:, :])
            nc.sync.dma_start(out=bx_sb[:, :], in_=bxrow_d[:, :])

            wih_sb = sb.tile([D_SEQ, 2 * H], FP8, tag="wih", name="wih")
            whp_sb = sb.tile([128, 2 * 2 * H], FP8, tag="whp", name="whp")
            ident_sb = sb.tile([128, 2 * BL], FP8, tag="ident", name="ident")
            invl_sb = sb.tile([128, LANES], FP32, tag="invl", name="invl")
            nc.sync.dma_start(out=wih_sb[:, :], in_=wih_d[:, :])
            nc.sync.dma_start(out=whp_sb[:, :], in_=whp_d[:, :])
            nc.sync.dma_start(out=ident_sb[:, :], in_=ident_d[:, :])
            nc.sync.dma_start(out=invl_sb[:, :], in_=invl_d[:, :])
            xtok_sb = [sb.tile([D_SEQ, max(S2[b], 2)], FP8, tag=f"xtok{b}", name=f"xtok{b}")
                       for b in range(2)]
            for b in range(2):
                for (t0, t1) in _dma_chunks([2 * w for w in wpad[b]]):
                    a0, a1 = int(off[b][t0]), int(off[b][t1])
                    if a1 > a0:
                        nc.sync.dma_start(out=xtok_sb[b][:, a0:a1], in_=xtok_d[b][:, a0:a1])

            # scan state: h double-buffered per block, fp8
            h_sb = [[sb.tile([128, HC * BL], FP8, tag=f"h{b}{p}", name=f"h{b}{p}")
                     for p in range(2)] for b in range(2)]
            po_sb = [sb.tile([128, HC * BL], FP32, tag=f"po{b}", name=f"po{b}")
                     for b in range(2)]
            xp_sb = [sb.tile([128, D_PROJ], FP32, tag=f"xp{bc}", name=f"xpsb{bc}")
                     for bc in range(2)]

            # pull the tanh table load into the DMA window (overwritten later)
            nc.scalar.activation(po_sb[0][:, 0:8], po_sb[0][:, 0:8], Tanh)

            # ---- PSUM: ps + pool-acc per block, 2 banks each = 8 banks ----
            ps_t = [psp.tile([128, HC * BL], FP32, tag=f"ps{b}", name=f"ps{b}")
                    for b in range(2)]
            acc_t = [psp.tile([128, HC * BL], FP32, tag=f"acc{b}", name=f"acc{b}")
                     for b in range(2)]

            # ---- phase A: x projection (PE ramp warm-up), fp32r ----
            # four [128, 512] pieces, each exactly one PSUM bank of ps_t
            for bc in range(2):
                for jh in range(2):
                    o = ps_t[bc][:, jh * H:(jh + 1) * H]
                    nc.tensor.matmul(o, ones_sb[0:1, :], bx_sb[0:1, jh * H:(jh + 1) * H],
                                     start=True, stop=False)
                    for kc in range(4):
                        nc.tensor.matmul(o, xT_sb[kc][:, bc * 128:(bc + 1) * 128],
                                         wx_sb[kc][:, jh * H:(jh + 1) * H],
                                         start=False, stop=(kc == 3))
                    nc.vector.tensor_copy(xp_sb[bc][:, jh * H:(jh + 1) * H], o)
                nc.sync.dma_start(out=xp_d[bc * 128:(bc + 1) * 128, :], in_=xp_sb[bc][:, :])

            # 3D views
            wih3 = wih_sb.rearrange("p (s j) -> p s j", s=2)       # [128, 2, 512]
            whp3 = whp_sb.rearrange("p (q s j) -> p q s j", q=2, s=2)  # [128, kp, 2, 512]
            id3 = ident_sb.rearrange("p (s j) -> p s j", s=2)      # [128, 2, 256]

            # ---- scan ----
            for t in range(steps):
                for b in range(2):
                    w = wpad[b][t]
                    if w == 0:
                        continue
                    a = int(off[b][t])
                    hw = h_sb[b][t % 2]
                    hr = h_sb[b][(t + 1) % 2]
                    ps3 = ps_t[b].rearrange("p (c n) -> p c n", c=HC)
                    acc3 = acc_t[b].rearrange("p (c n) -> p c n", c=HC)
                    hw3 = hw.rearrange("p (c n) -> p c n", c=HC)
                    hr3 = hr.rearrange("p (c n) -> p c n", c=HC)
                    xt3 = xtok_sb[b][:, a:a + 2 * w].rearrange("p (s n) -> p s n", s=2)

                    # input proj + masked bias: 4 DR matmuls
                    for jc in range(HC):
                        nc.tensor.matmul(ps3[:, jc, 0:w],
                                         wih3[:, :, jc * 128:(jc + 1) * 128], xt3,
                                         start=(jc % 2 == 0), stop=(t == 0),
                                         perf_mode=DR, skip_group_check=True)
                    if t > 0:
                        # recurrence: 2 k-pairs x 4 j-chunks
                        for p in range(2):
                            hk = hr3[:, 2 * p:2 * p + 2, 0:w]
                            for jc in range(HC):
                                nc.tensor.matmul(ps3[:, jc, 0:w],
                                                 whp3[:, p, :, jc * 128:(jc + 1) * 128], hk,
                                                 start=False, stop=(p == 1),
                                                 perf_mode=DR, skip_group_check=True)
                        # mean-pool accumulate h_{t-1}: 4 identity DR matmuls
                        wp = wtrue[b][t - 1]
                        if wp > 0:
                            for p in range(2):
                                hk = hr3[:, 2 * p:2 * p + 2, 0:wp]
                                for s in range(2):
                                    jc = 2 * p + s
                                    nc.tensor.matmul(acc3[:, jc, 0:wp],
                                                     id3[:, :, s * 128:(s + 1) * 128], hk,
                                                     start=(t == 1 and s == 0), stop=False,
                                                     perf_mode=DR, skip_group_check=True)

                    nc.scalar.activation(hw3[:, 0:HC, 0:w], ps3[:, 0:HC, 0:w],
                                         Tanh, scale=1.0 / LAM)

            # final pool pass: h of the last step per block
            for b in range(2):
                t_last = max(t for t in range(steps) if wpad[b][t] > 0)
                wp = wtrue[b][t_last]
                hl3 = h_sb[b][t_last % 2].rearrange("p (c n) -> p c n", c=HC)
                acc3 = acc_t[b].rearrange("p (c n) -> p c n", c=HC)
                for p in range(2):
                    hk = hl3[:, 2 * p:2 * p + 2, 0:wp]
                    for s in range(2):
                        jc = 2 * p + s
                        nc.tensor.matmul(acc3[:, jc, 0:wp],
                                         id3[:, :, s * 128:(s + 1) * 128], hk,
                                         start=(steps == 1 and s == 0), stop=True,
                                         perf_mode=DR, skip_group_check=True)

            # ---- finalize: pooledT = acc * (1/len) ----
            for b in range(2):
                acc3 = acc_t[b].rearrange("p (c n) -> p c n", c=HC)
                po3 = po_sb[b].rearrange("p (c n) -> p c n", c=HC)
                for jc in range(HC):
                    nc.vector.tensor_mul(po3[:, jc, :], acc3[:, jc, :],
                                         invl_sb[:, b * BL:(b + 1) * BL])
                    nc.sync.dma_start(out=pooledT_d[b][jc * 128:(jc + 1) * 128, :],
                                      in_=po_sb[b][:, jc * BL:(jc + 1) * BL])

    nc.compile()
    return nc


def _get_program(sched_key, sched):
    if sched_key not in _program_cache:
        _program_cache[sched_key] = _build_program(sched)
    return _program_cache[sched_key]


def _prepare(x, seqs, masks, W_ih_f, W_hh_f, b_f, W_ih_b, W_hh_b, b_b, Wx, bx):
    x = np.asarray(x, np.float32)
    seqs = np.asarray(seqs, np.float32)
    masks = np.asarray(masks).astype(np.int64)

    # ---- segment geometry (host) ----
    lens = np.bincount(masks, minlength=N_GROUPS).astype(np.int64)
    starts_all = np.concatenate([[0], np.cumsum(lens)[:-1]])
    order = np.argsort(-lens, kind="stable")
    sl = lens[order]
    L = sl[0::4].astype(np.int64)                 # shared lane schedule (512)
    steps = int(L[0])

    # block split: even/odd lane ranks
    Lb = [L[0::2], L[1::2]]                       # [2][BL]
    wtrue = [[int((Lb[b] > t).sum()) for t in range(steps)] for b in range(2)]
    wpad = [[min(BL, (w + 3) // 4 * 4) if w > 0 else 0 for w in wtrue[b]]
            for b in range(2)]
    off = []
    for b in range(2):
        off.append(np.concatenate([[0], np.cumsum([2 * w for w in wpad[b]])]).astype(int))
    sched = (tuple(tuple(v) for v in wtrue), tuple(tuple(v) for v in wpad))
    sched_key = sched

    seqs_pad = np.vstack([np.zeros((1, D_SEQ), np.float32), seqs])

    gid = [order[c::4] for c in range(4)]
    t_grid = np.arange(steps)[:, None]

    per_stripe = {}
    for c in range(4):
        lens_c = lens[gid[c]]
        starts_c = starts_all[gid[c]]
        blk = {}
        for b in range(2):
            lens_cb = lens_c[b::2]
            starts_cb = starts_c[b::2]
            Lb_ = Lb[b][None, :]
            pre = Lb_ - lens_cb[None, :]
            real = (t_grid < Lb_) & (t_grid >= pre)
            pos = t_grid - pre
            idx_f = np.where(real, starts_cb[None, :] + pos, -1)
            idx_b = np.where(real, starts_cb[None, :] + lens_cb[None, :] - 1 - pos, -1)
            S2 = int(off[b][-1])
            xtf = np.zeros((D_SEQ, S2), np.float32)
            xtb = np.zeros((D_SEQ, S2), np.float32)
            for t in range(steps):
                w = wpad[b][t]
                if w == 0:
                    continue
                wt = wtrue[b][t]
                a = int(off[b][t])
                # slab0: tokens
                xtf[:, a:a + wt] = seqs_pad[idx_f[t, :wt] + 1].T
                xtb[:, a:a + wt] = seqs_pad[idx_b[t, :wt] + 1].T
                # slab1: mask row 0, zeros elsewhere
                m = real[t, :wt].astype(np.float32)
                xtf[0, a + w:a + w + wt] = m
                xtb[0, a + w:a + w + wt] = m
            invl = (1.0 / np.maximum(lens_cb, 1)).astype(np.float32)
            blk[b] = (xtf.astype(NP8), xtb.astype(NP8), invl)
        per_stripe[c] = blk

    def quant(a):
        return np.ascontiguousarray(a).astype(NP8)

    def wih_pack(W_ih, bvec):
        out = np.zeros((D_SEQ, 2 * H), np.float32)
        out[:, 0:H] = LAM * np.asarray(W_ih, np.float32).T
        out[0, H:2 * H] = LAM * np.asarray(bvec, np.float32)
        return quant(out)

    def whp_pack(W_hh):
        WT = LAM * np.asarray(W_hh, np.float32).T    # [512 k, 512 j]
        out = np.zeros((128, 4 * H), np.float32)
        for p in range(2):
            for s in range(2):
                kc = 2 * p + s
                out[:, (2 * p + s) * H:(2 * p + s + 1) * H] = WT[kc * 128:(kc + 1) * 128, :]
        return quant(out)

    ident = np.zeros((128, 2 * BL), np.float32)
    ident[:, 0:128] = np.eye(128)
    ident[:, BL + 128:BL + 256] = np.eye(128)
    ident = quant(ident)

    ones = np.ones((1, 128), np.float32)
    wxT = np.ascontiguousarray(np.asarray(Wx, np.float32).T)
    bxr = np.asarray(bx, np.float32)[None, :]

    in_maps = []
    for core in range(N_CORES):
        c = core % 4
        fwd = core < 4
        blk = per_stripe[c]
        invl_full = np.zeros((128, LANES), np.float32)
        invl_full[:, 0:BL] = blk[0][2][None, :]
        invl_full[:, BL:2 * BL] = blk[1][2][None, :]
        in_maps.append({
            "xtok0": blk[0][0] if fwd else blk[0][1],
            "xtok1": blk[1][0] if fwd else blk[1][1],
            "wih": wih_pack(W_ih_f if fwd else W_ih_b, b_f if fwd else b_b),
            "whp": whp_pack(W_hh_f if fwd else W_hh_b),
            "ident": ident,
            "invl": invl_full,
            "xT": np.ascontiguousarray(x[core * XROWS:(core + 1) * XROWS, :].T),
            "wxT": wxT,
            "bxrow": bxr,
            "ones": ones,
        })

    return (sched_key, sched), in_maps, gid


def _assemble(res, gid):
    out = np.empty((N_GROUPS, 2 * D_PROJ), np.float32)
    for core in range(N_CORES):
        out[core * XROWS:(core + 1) * XROWS, :D_PROJ] = res[core]["xp"]
    for c in range(4):
        for b in range(2):
            g = gid[c][b::2]
            out[g, D_PROJ:D_PROJ + H] = res[c][f"pooledT{b}"].T
            out[g, D_PROJ + H:] = res[c + 4][f"pooledT{b}"].T
    return out


def kernel(**inputs):
    (sched_key, sched), in_maps, gid = _prepare(**inputs)
    nc = _get_program(sched_key, sched)
    res = run_bass_kernel_spmd(nc, in_maps, list(range(N_CORES))).results
    return _assemble(res, gid)


# revision 7
# speedup vs baseline: 1.9214x; 1.1891x over previous
"""Trainium2 Bass kernel: GSpade node embedding (fp8 DoubleRow scan).

Computation (see reference):
  - bidirectional tanh-RNN (hidden 512/dir) over T=32768 tokens grouped into
    N=2048 contiguous ragged segments (sorted group ids in `masks`)
  - mean-pool hidden states per segment -> pooled [N, 1024]
  - out = [x @ Wx.T + bx | pooled]  -> [N, 2048]

Sharding (8 NeuronCores, SPMD single program):
  cores 0-3 forward RNN, 4-7 backward.  Segments sorted by length (desc) and
  striped: stripe c (= core % 4) owns segments ranked 4i+c.  Each core also
  computes 256 rows of the x-projection.

Per-core scan: h kept transposed [feature(4x128p), lane].  Lanes end-aligned
to the shared schedule L_i (max of the 4 stripes at rank i) with zero-prefix,
so h stays exactly 0 until the first real token (bias is masked via a second
matmul k-slab).  Lanes are split into two interleaved blocks (even/odd rank)
that alternate per step: while ACT tanh's block A, the PE runs block B's
matmuls - hides the tanh latency in the serial recurrence chain.

All scan matmuls are fp8e4 DoubleRow (2 k-slabs per instr, 0.5 PE cycles per
output column):
  - input proj: slab0 = 64*W_ih.T vs token slab, slab1 = [64*b; 0..] vs
    [mask; 0..] slab -> psum = 64*(W_ih.T x + b*mask) in one pass
  - recurrence: 2 k-pair DR matmuls per j-chunk vs fp8 h of prev step
  - mean-pool:  [I|0] / [0|I] identity DR matmuls accumulate h into a
    PSUM accumulator (start at t=1, stop after the last step)
ACT: h = tanh(psum * 1/64) -> fp8, one instr per block-step.
PSUM: 2 banks ps + 2 banks pool-acc per block = all 8 banks.

Phase order minimizes serialized DMA (HWDGE ~630ns/dma is a shared resource):
fp8 pack + stream chunks load first (scan starts ~1.5us in); x-projection
weights (bf16) stream during the scan; xproj runs at the end on the idle PE,
reusing the freed ps banks.  Pooling finalizes in two batches (retired lanes
mid-scan, remainder in the tail), one rearranged-AP DMA per block.
"""

import ml_dtypes
import numpy as np

import concourse.bacc as bacc
import concourse.mybir as mybir
from concourse.tile import TileContext
from concourse.bass_utils import run_bass_kernel_spmd

FP32 = mybir.dt.float32
F32R = mybir.dt.float32r
BF16 = mybir.dt.bfloat16
FP8 = mybir.dt.float8e4
NP8 = ml_dtypes.float8_e4m3
NPB = ml_dtypes.bfloat16
Tanh = mybir.ActivationFunctionType.Tanh
DR = mybir.MatmulPerfMode.DoubleRow

N_GROUPS = 2048
D_SEQ = 128
H = 512           # hidden per direction
HC = 4            # hidden chunks of 128
D_PROJ = 1024
N_CORES = 8
LANES = 512       # segments per scan core
BL = 256          # lanes per block
XROWS = N_GROUPS // N_CORES  # x-projection rows per core
LAM = 64.0        # fp8 weight scale, undone by ACT scale=1/LAM

_program_cache: dict = {}


def _dma_chunks(widths, target=2048):
    chunks, t0, cols = [], 0, 0
    for t, w in enumerate(widths):
        if cols > 0 and cols + w > target:
            chunks.append((t0, t))
            t0, cols = t, 0
        cols += w
    chunks.append((t0, len(widths)))
    return chunks


def _build_program(sched):
    """sched = (wtrue[2][steps], wpad[2][steps]) per block."""
    wtrue, wpad = sched
    steps = len(wtrue[0])
    off = []
    for b in range(2):
        o = np.concatenate([[0], np.cumsum([2 * w for w in wpad[b]])]).astype(int)
        off.append(o)
    S2 = [int(off[b][-1]) for b in range(2)]
    # mid-scan finalize point per block: first step with width <= BL/2
    t_half = []
    for b in range(2):
        cand = [t for t in range(steps) if wtrue[b][t] <= BL // 2]
        t_half.append(cand[0] if cand else steps)

    nc = bacc.Bacc("TRN2", target_bir_lowering=False, debug=False,
                   num_devices=N_CORES)

    # fp8 pack: wih [128,1024] | whp [128,2048] | ident [128,512]
    pk8_d = nc.dram_tensor("pk8", [128, 3584], FP8, kind="ExternalInput")
    xtok_d = [nc.dram_tensor(f"xtok{b}", [D_SEQ, max(S2[b], 2)], FP8,
                             kind="ExternalInput") for b in range(2)]
    invl_d = nc.dram_tensor("invl", [128, LANES], FP32, kind="ExternalInput")
    onesbx_d = nc.dram_tensor("onesbx", [1, 128 + D_PROJ], F32R, kind="ExternalInput")
    xTb_d = nc.dram_tensor("xTb", [128, 4 * XROWS], BF16, kind="ExternalInput")
    wxb_d = nc.dram_tensor("wxb", [128, 4 * D_PROJ], BF16, kind="ExternalInput")

    xp_d = nc.dram_tensor("xp", [XROWS, D_PROJ], FP32, kind="ExternalOutput")
    pooledT_d = [nc.dram_tensor(f"pooledT{b}", [H, BL], FP32,
                                kind="ExternalOutput") for b in range(2)]

    with TileContext(nc) as tc:
        with (
            tc.tile_pool(name="sb", bufs=1) as sb,
            tc.tile_pool(name="ps", bufs=1, space="PSUM") as psp,
        ):
            # ---- SBUF tiles ----
            pk8_sb = sb.tile([128, 3584], FP8, tag="pk8", name="pk8")
            xtok_sb = [sb.tile([D_SEQ, max(S2[b], 2)], FP8, tag=f"xtok{b}", name=f"xtok{b}")
                       for b in range(2)]
            invl_sb = sb.tile([128, LANES], FP32, tag="invl", name="invl")
            onesbx_sb = sb.tile([1, 128 + D_PROJ], F32R, tag="onesbx", name="onesbx")
            xTb_sb = sb.tile([128, 4 * XROWS], BF16, tag="xTb", name="xTb")
            wxb_sb = sb.tile([128, 4 * D_PROJ], BF16, tag="wxb", name="wxb")
            h_sb = [[sb.tile([128, HC * BL], FP8, tag=f"h{b}{p}", name=f"h{b}{p}")
                     for p in range(2)] for b in range(2)]
            po_sb = [sb.tile([128, HC * BL], FP32, tag=f"po{b}", name=f"po{b}")
                     for b in range(2)]
            xp_sb = [sb.tile([128, D_PROJ], FP32, tag=f"xp{bc}", name=f"xpsb{bc}")
                     for bc in range(2)]

            # ---- DMA order: scan deps first, xproj weights during scan ----
            nc.sync.dma_start(out=pk8_sb[:, :], in_=pk8_d[:, :])
            chunked = [[(int(off[b][t0]), int(off[b][t1]))
                        for (t0, t1) in _dma_chunks([2 * w for w in wpad[b]])]
                       for b in range(2)]
            ml = max(len(chunked[0]), len(chunked[1]))
            for i in range(ml):
                for b in range(2):
                    if i < len(chunked[b]):
                        a0, a1 = chunked[b][i]
                        if a1 > a0:
                            nc.sync.dma_start(out=xtok_sb[b][:, a0:a1],
                                              in_=xtok_d[b][:, a0:a1])
            nc.sync.dma_start(out=invl_sb[:, :], in_=invl_d[:, :])
            nc.sync.dma_start(out=onesbx_sb[:, :], in_=onesbx_d[:, :])
            nc.sync.dma_start(out=xTb_sb[:, :], in_=xTb_d[:, :])
            nc.sync.dma_start(out=wxb_sb[:, :], in_=wxb_d[:, :])

            # pull the tanh table load ahead of the scan
            nc.scalar.activation(po_sb[0][:, 0:8], po_sb[0][:, 0:8], Tanh)

            # ---- PSUM: ps + pool-acc per block, 2 banks each = 8 banks ----
            ps_t = [psp.tile([128, HC * BL], FP32, tag=f"ps{b}", name=f"ps{b}")
                    for b in range(2)]
            acc_t = [psp.tile([128, HC * BL], FP32, tag=f"acc{b}", name=f"acc{b}")
                     for b in range(2)]

            # 3D weight views
            wih3 = pk8_sb[:, 0:1024].rearrange("p (s j) -> p s j", s=2)
            whp4 = pk8_sb[:, 1024:3072].rearrange("p (q s j) -> p q s j", q=2, s=2)
            id3 = pk8_sb[:, 3072:3584].rearrange("p (s j) -> p s j", s=2)

            def pool_pass(b, hsrc3, wp, start, stop):
                acc3 = acc_t[b].rearrange("p (c n) -> p c n", c=HC)
                for p in range(2):
                    hk = hsrc3[:, 2 * p:2 * p + 2, 0:wp]
                    for s in range(2):
                        jc = 2 * p + s
                        nc.tensor.matmul(acc3[:, jc, 0:wp],
                                         id3[:, :, s * 128:(s + 1) * 128], hk,
                                         start=(start and s == 0), stop=stop,
                                         perf_mode=DR, skip_group_check=True)

            def finalize(b, lo, hi):
                acc3 = acc_t[b].rearrange("p (c n) -> p c n", c=HC)
                po3 = po_sb[b].rearrange("p (c n) -> p c n", c=HC)
                for jc in range(HC):
                    nc.vector.tensor_mul(po3[:, jc, lo:hi], acc3[:, jc, lo:hi],
                                         invl_sb[:, b * BL + lo:b * BL + hi])
                o3 = pooledT_d[b][:, :].rearrange("(c p) n -> p c n", c=HC)
                nc.sync.dma_start(out=o3[:, :, lo:hi], in_=po3[:, :, lo:hi])

            # ---- scan ----
            for t in range(steps):
                for b in range(2):
                    w = wpad[b][t]
                    if w == 0:
                        continue
                    a = int(off[b][t])
                    hw = h_sb[b][t % 2]
                    hr = h_sb[b][(t + 1) % 2]
                    ps3 = ps_t[b].rearrange("p (c n) -> p c n", c=HC)
                    hw3 = hw.rearrange("p (c n) -> p c n", c=HC)
                    hr3 = hr.rearrange("p (c n) -> p c n", c=HC)
                    xt3 = xtok_sb[b][:, a:a + 2 * w].rearrange("p (s n) -> p s n", s=2)

                    # input proj + masked bias: 4 DR matmuls
                    for jc in range(HC):
                        nc.tensor.matmul(ps3[:, jc, 0:w],
                                         wih3[:, :, jc * 128:(jc + 1) * 128], xt3,
                                         start=(jc % 2 == 0), stop=(t == 0),
                                         perf_mode=DR, skip_group_check=True)
                    if t > 0:
                        for p in range(2):
                            hk = hr3[:, 2 * p:2 * p + 2, 0:w]
                            for jc in range(HC):
                                nc.tensor.matmul(ps3[:, jc, 0:w],
                                                 whp4[:, p, :, jc * 128:(jc + 1) * 128], hk,
                                                 start=False, stop=(p == 1),
                                                 perf_mode=DR, skip_group_check=True)
                        wp = wtrue[b][t - 1]
                        if wp > 0:
                            pool_pass(b, hr3, wp, start=(t == 1), stop=False)

                    nc.scalar.activation(hw3[:, 0:HC, 0:w], ps3[:, 0:HC, 0:w],
                                         Tanh, scale=1.0 / LAM)

                    # mid-scan finalize of retired lanes [w_half:BL]
                    if t == t_half[b]:
                        finalize(b, wtrue[b][t], BL)

            # final pool pass + tail finalize
            for b in range(2):
                t_last = max(tt for tt in range(steps) if wpad[b][tt] > 0)
                wp = wtrue[b][t_last]
                hl3 = h_sb[b][t_last % 2].rearrange("p (c n) -> p c n", c=HC)
                pool_pass(b, hl3, wp, start=(steps == 1), stop=True)
                finalize(b, 0, wtrue[b][t_half[b]] if t_half[b] < steps else BL)

            # ---- x projection (tail, PE idle, ps banks free), bf16 ----
            ones_ap = onesbx_sb[0:1, 0:128]
            for bc in range(2):
                for jh in range(2):
                    o = ps_t[bc][:, jh * H:(jh + 1) * H]
                    nc.tensor.matmul(o, ones_ap,
                                     onesbx_sb[0:1, 128 + jh * H:128 + (jh + 1) * H],
                                     start=True, stop=False)
                    for kc in range(4):
                        nc.tensor.matmul(o, xTb_sb[:, kc * XROWS + bc * 128:kc * XROWS + (bc + 1) * 128],
                                         wxb_sb[:, kc * D_PROJ + jh * H:kc * D_PROJ + (jh + 1) * H],
                                         start=False, stop=(kc == 3))
                    nc.vector.tensor_copy(xp_sb[bc][:, jh * H:(jh + 1) * H], o)
                nc.sync.dma_start(out=xp_d[bc * 128:(bc + 1) * 128, :], in_=xp_sb[bc][:, :])

    nc.compile()
    return nc


def _get_program(sched_key, sched):
    if sched_key not in _program_cache:
        _program_cache[sched_key] = _build_program(sched)
    return _program_cache[sched_key]


def _prepare(x, seqs, masks, W_ih_f, W_hh_f, b_f, W_ih_b, W_hh_b, b_b, Wx, bx):
    x = np.asarray(x, np.float32)
    seqs = np.asarray(seqs, np.float32)
    masks = np.asarray(masks).astype(np.int64)

    lens = np.bincount(masks, minlength=N_GROUPS).astype(np.int64)
    starts_all = np.concatenate([[0], np.cumsum(lens)[:-1]])
    order = np.argsort(-lens, kind="stable")
    sl = lens[order]
    L = sl[0::4].astype(np.int64)
    steps = int(L[0])

    Lb = [L[0::2], L[1::2]]
    wtrue = [[int((Lb[b] > t).sum()) for t in range(steps)] for b in range(2)]
    wpad = [[min(BL, (w + 3) // 4 * 4) if w > 0 else 0 for w in wtrue[b]]
            for b in range(2)]
    off = []
    for b in range(2):
        off.append(np.concatenate([[0], np.cumsum([2 * w for w in wpad[b]])]).astype(int))
    sched = (tuple(tuple(v) for v in wtrue), tuple(tuple(v) for v in wpad))
    sched_key = sched

    seqs_pad = np.vstack([np.zeros((1, D_SEQ), np.float32), seqs])
    gid = [order[c::4] for c in range(4)]
    t_grid = np.arange(steps)[:, None]

    per_stripe = {}
    for c in range(4):
        lens_c = lens[gid[c]]
        starts_c = starts_all[gid[c]]
        blk = {}
        for b in range(2):
            lens_cb = lens_c[b::2]
            starts_cb = starts_c[b::2]
            Lb_ = Lb[b][None, :]
            pre = Lb_ - lens_cb[None, :]
            real = (t_grid < Lb_) & (t_grid >= pre)
            pos = t_grid - pre
            idx_f = np.where(real, starts_cb[None, :] + pos, -1)
            idx_b = np.where(real, starts_cb[None, :] + lens_cb[None, :] - 1 - pos, -1)
            S2 = int(off[b][-1])
            xtf = np.zeros((D_SEQ, S2), np.float32)
            xtb = np.zeros((D_SEQ, S2), np.float32)
            for t in range(steps):
                w = wpad[b][t]
                if w == 0:
                    continue
                wt = wtrue[b][t]
                a = int(off[b][t])
                xtf[:, a:a + wt] = seqs_pad[idx_f[t, :wt] + 1].T
                xtb[:, a:a + wt] = seqs_pad[idx_b[t, :wt] + 1].T
                m = real[t, :wt].astype(np.float32)
                xtf[0, a + w:a + w + wt] = m
                xtb[0, a + w:a + w + wt] = m
            invl = (1.0 / np.maximum(lens_cb, 1)).astype(np.float32)
            blk[b] = (xtf.astype(NP8), xtb.astype(NP8), invl)
        per_stripe[c] = blk

    def q8(a):
        return np.ascontiguousarray(a).astype(NP8)

    def pk8_pack(W_ih, bvec, W_hh):
        out = np.zeros((128, 3584), np.float32)
        out[:, 0:H] = LAM * np.asarray(W_ih, np.float32).T
        out[0, H:2 * H] = LAM * np.asarray(bvec, np.float32)
        WT = LAM * np.asarray(W_hh, np.float32).T
        for kc in range(4):
            out[:, 1024 + kc * H:1024 + (kc + 1) * H] = WT[kc * 128:(kc + 1) * 128, :]
        out[:, 3072:3200] = np.eye(128)
        out[:, 3456:3584] = np.eye(128)
        return q8(out)

    pk8 = {
        0: pk8_pack(W_ih_f, b_f, W_hh_f),
        1: pk8_pack(W_ih_b, b_b, W_hh_b),
    }

    onesbx = np.zeros((1, 128 + D_PROJ), np.float32)
    onesbx[0, 0:128] = 1.0
    onesbx[0, 128:] = np.asarray(bx, np.float32)
    wxT = np.asarray(Wx, np.float32).T                     # [512, 1024]
    wxb = np.concatenate([wxT[kc * 128:(kc + 1) * 128, :] for kc in range(4)],
                         axis=1).astype(NPB)               # [128, 4096]

    in_maps = []
    for core in range(N_CORES):
        c = core % 4
        fwd = core < 4
        blk = per_stripe[c]
        invl_full = np.zeros((128, LANES), np.float32)
        invl_full[:, 0:BL] = blk[0][2][None, :]
        invl_full[:, BL:2 * BL] = blk[1][2][None, :]
        xT = x[core * XROWS:(core + 1) * XROWS, :].T       # [512, 256]
        xTb = np.concatenate([xT[kc * 128:(kc + 1) * 128, :] for kc in range(4)],
                             axis=1).astype(NPB)           # [128, 1024]
        in_maps.append({
            "pk8": pk8[0 if fwd else 1],
            "xtok0": blk[0][0] if fwd else blk[0][1],
            "xtok1": blk[1][0] if fwd else blk[1][1],
            "invl": invl_full,
            "onesbx": onesbx,
            "xTb": xTb,
            "wxb": wxb,
        })

    return (sched_key, sched), in_maps, gid


def _assemble(res, gid):
    out = np.empty((N_GROUPS, 2 * D_PROJ), np.float32)
    for core in range(N_CORES):
        out[core * XROWS:(core + 1) * XROWS, :D_PROJ] = res[core]["xp"]
    for c in range(4):
        for b in range(2):
            g = gid[c][b::2]
            out[g, D_PROJ:D_PROJ + H] = res[c][f"pooledT{b}"].T
            out[g, D_PROJ + H:] = res[c + 4][f"pooledT{b}"].T
    return out


def kernel(**inputs):
    (sched_key, sched), in_maps, gid = _prepare(**inputs)
    nc = _get_program(sched_key, sched)
    res = run_bass_kernel_spmd(nc, in_maps, list(range(N_CORES))).results
    return _assemble(res, gid)
